# revision 1
# baseline (speedup 1.0000x reference)
"""Binary 3D dilation (star/6-connected structuring element) on 8 TRN2 cores.

out = (conv3d(x, star_kernel, pad=1) > 0)  for x in {0,1}^(2,1,256,256,256)

Decomposition per output voxel:
    s = (x[d-1] + x[d] + x[d+1])          # Z: D-axis 3-sum (incl center)
      + (x[h-1] + x[h+1])                 # H shifts
      + (x[w-1] + x[w+1])                 # a: W shifts
    out = sign(s)            (s >= 0, integer-valued, so sign == (s > 0))

Sharding: core k -> batch k//4, D-quarter k%4. Each core receives a
66-plane slab (64 output planes + 1 halo plane each side, zero-padded at
the volume boundary), so no cross-core communication is needed.

Number formats: values are 0/1 (sums <= 7), so fp8e4m3 / bf16 are exact.
DRAM I/O is fp8 with a partition-major layout (multi-KB DMA descriptor
runs); SBUF compute is bf16 (DVE 2x mode); the DMA casts in flight.

H-INTERLEAVED partition layout: partition p, sub-column c in {0,1} holds
row h = 2p + c. The H-stencil then needs rows from the OTHER parity at
partitions p-1,p / p,p+1 -- two bidiagonal matmuls whose matrix edges
encode the h=0/255 boundaries exactly, so there is no chunk-boundary
halo at all:
  PE   : psum[c0] = B0^T @ x[d,c1]   (B0[k,m] = [k in {m-1,m}])
         psum[c1] = B1^T @ x[d,c0]   (B1[k,m] = [k in {m,m+1}])
         psum    += I^T @ g          (g = a + Z, merged on DVE)
                 (or += I^T @ a + I^T @ Z for PE/DVE balance)
  DVE  : a = x[w-1]+x[w+1]; Z = x[d-1]+x[d+1] then += x[d]; g = a+Z
         (all block-granular over PBLK planes)
  ACT  : out = Sign(psum), batched over PBLK planes
  DMA  : overlapped block loads / stores (SWDGE, fp8<->bf16 cast).
"""

import sys

import numpy as np

if "/opt/trn_rl_repo" not in sys.path:
    sys.path.insert(0, "/opt/trn_rl_repo")

B = 2
D_TOT = 256
H = 256
W = 256
N_CORES = 8
D_SHARDS = 4             # D split per batch entry
D_OUT = D_TOT // D_SHARDS          # 64 output planes per core
D_IN = D_OUT + 2                   # + halo plane each side
PBLK = 4                           # output planes per block
IBLK = PBLK + 2                    # input planes per block (overlapped)
WP = W + 2                         # host-padded row width (zero cols 0, 257)
MERGE_K = 2                        # planes per block with g=a+Z merged on DVE

# 6-connected "star" structuring element mask (D,H,W offsets from center)
_STAR = np.zeros((3, 3, 3), bool)
_STAR[1, 1, 1] = _STAR[0, 1, 1] = _STAR[2, 1, 1] = True
_STAR[1, 0, 1] = _STAR[1, 2, 1] = True
_STAR[1, 1, 0] = _STAR[1, 1, 2] = True

# extra kwargs for run_bass_kernel_spmd (test.py sets trace=True here)
RUN_KWARGS: dict = {}
LAST_RESULTS = None


def _b0() -> np.ndarray:
    m = np.zeros((128, 128), np.float32)
    i = np.arange(128)
    m[i, i] = 1.0
    m[i[:-1], i[:-1] + 1] = 1.0   # k = m-1
    return m


def _b1() -> np.ndarray:
    m = np.zeros((128, 128), np.float32)
    i = np.arange(128)
    m[i, i] = 1.0
    m[i[1:], i[1:] - 1] = 1.0     # k = m+1
    return m


def build_nc(d_out: int = D_OUT, merge_k: int = MERGE_K):
    """Build the per-core Bass program (identical on all cores)."""
    import concourse.bass as bass
    import concourse.mybir as mybir
    import concourse.tile as tile

    f32 = mybir.dt.float32
    bf16 = mybir.dt.bfloat16
    fp8 = mybir.dt.float8e4

    d_in = d_out + 2
    assert d_out % PBLK == 0
    # small blocks at the ends shorten the pipeline head (first compute
    # starts after a smaller load) and tail (shorter mm->sign->store drain)
    if d_out >= 3 * PBLK:
        # middle blocks alternate merge_k 1/2 to balance DVE vs PE
        blocks = [(2, 1), (2, 1)]
        blocks += [(PBLK, 1 + (i % 2)) for i in range((d_out - 8) // PBLK)]
        blocks += [(2, 1), (2, 1)]
    else:
        blocks = [(PBLK, merge_k)] * (d_out // PBLK)

    nc = bass.Bass()
    # partition-major DRAM layouts: [p, plane, c, w(padded)], h = 2p + c
    x = nc.declare_dram_parameter("x", [128, d_in, 2, WP], fp8, isOutput=False)
    b0_d = nc.declare_dram_parameter("b0", [128, 128], bf16, isOutput=False)
    b1_d = nc.declare_dram_parameter("b1", [128, 128], bf16, isOutput=False)
    eye_d = nc.declare_dram_parameter("eye", [128, 128], bf16, isOutput=False)
    y = nc.declare_dram_parameter("y", [128, d_out, 2, W], fp8, isOutput=True)

    with tile.TileContext(nc) as tc:
        with (
            tc.tile_pool(name="consts", bufs=1) as cpool,
            tc.tile_pool(name="inblk", bufs=8) as ipool,
            tc.tile_pool(name="outblk", bufs=4) as opool,
            tc.tile_pool(name="awork", bufs=6) as apool,
            tc.tile_pool(name="psum", bufs=2, space=bass.MemorySpace.PSUM) as ppool,
        ):
            b0 = cpool.tile([128, 128], bf16, tag="b0")
            b1 = cpool.tile([128, 128], bf16, tag="b1")
            eye = cpool.tile([128, 128], bf16, tag="eye")
            nc.sync.dma_start(out=b0[:], in_=b0_d[:])
            nc.sync.dma_start(out=b1[:], in_=b1_d[:])
            nc.sync.dma_start(out=eye[:], in_=eye_d[:])

            p0 = 0
            for npl, mk in blocks:
                # ---- load (fp8 DRAM -> bf16 SBUF, cast in DMA) ------------
                blk = ipool.tile([128, npl + 2, 2, WP], bf16, tag="in")
                nc.gpsimd.dma_start(out=blk[:], in_=x[:, p0 : p0 + npl + 2])
                # ---- block-granular elementwise (DVE) ---------------------
                # aw[:, p, c, 0] = a = x[d,w-1] + x[d,w+1]
                # aw[:, p, c, 1] = Z = x[d-1] + x[d] + x[d+1]
                aw = apool.tile([128, npl, 2, 2, W], bf16, tag="aw")
                cur = blk[:, 1 : 1 + npl]
                av = aw[:, :, :, 0]
                zv = aw[:, :, :, 1]
                nc.vector.tensor_add(
                    out=zv,
                    in0=blk[:, 0:npl, :, 1 : W + 1],
                    in1=blk[:, 2 : 2 + npl, :, 1 : W + 1],
                )
                nc.vector.tensor_add(
                    out=zv, in0=zv, in1=cur[:, :, :, 1 : W + 1]
                )
                nc.vector.tensor_add(
                    out=av, in0=cur[:, :, :, 0:W], in1=cur[:, :, :, 2 : 2 + W]
                )
                if mk:
                    # g = a + Z for the LAST mk planes (into j=0) -- the
                    # unmerged planes' matmuls don't wait on this, so PE
                    # starts while the merge finishes
                    nc.vector.tensor_add(
                        out=aw[:, npl - mk :, :, 0],
                        in0=aw[:, npl - mk :, :, 0],
                        in1=aw[:, npl - mk :, :, 1],
                    )
                # ---- matmuls + one batched sign ---------------------------
                ot = opool.tile([128, npl, 2, W], fp8, tag="out")
                ps = ppool.tile([128, npl, 2 * W], f32, tag="ps")
                for dd in range(npl):
                    merged = dd >= npl - mk
                    # full-span matmul first: start=True zeroes the whole
                    # bank, later matmuls accumulate. For unmerged planes
                    # lead with eye@Z (ready earliest in the DVE chain).
                    nc.tensor.matmul(
                        ps[:, dd],
                        eye[:],
                        aw[:, dd, :, 1 - int(merged)],
                        start=True,
                        stop=False,
                        skip_group_check=True,
                    )
                    nc.tensor.matmul(
                        ps[:, dd, 0:W],
                        b0[:],
                        cur[:, dd, 1, 1 : W + 1],
                        start=False,
                        stop=False,
                        skip_group_check=True,
                    )
                    nc.tensor.matmul(
                        ps[:, dd, W : 2 * W],
                        b1[:],
                        cur[:, dd, 0, 1 : W + 1],
                        start=False,
                        stop=merged,
                        skip_group_check=True,
                    )
                    if not merged:
                        nc.tensor.matmul(
                            ps[:, dd],
                            eye[:],
                            aw[:, dd, :, 0],
                            start=False,
                            stop=True,
                            skip_group_check=True,
                        )
                nc.scalar.sign(
                    out=ot[:].rearrange("h p c w -> h (p c w)"),
                    in_=ps[:].rearrange("h p n -> h (p n)"),
                )
                # ---- store (fp8 SBUF -> fp8 DRAM, HWDGE) ------------------
                nc.sync.dma_start(out=y[:, p0 : p0 + npl], in_=ot[:])
                p0 += npl

    # Walrus codegen allows at most 1 semaphore wait per engine instruction
    # (2 on InstEventSemaphore). Run the bacc passes that enforce this.
    import bass_rust as _bass_rust

    _bass_rust.move_matmul_waits_to_ldweights(nc.m)
    _bass_rust.generate_event_semaphores(nc)
    return nc


_NC_CACHE = None


def host_inputs(slab_f32: np.ndarray) -> dict:
    """Per-core in_map from a zero-padded (d_in, H, WP) slab (0/1 values)."""
    import ml_dtypes

    f8 = ml_dtypes.float8_e4m3fn
    d_in = slab_f32.shape[0]
    # [plane, (p c), w] -> [p, plane, c, w]   (h = 2p + c interleave)
    xh = np.ascontiguousarray(
        slab_f32.reshape(d_in, 128, 2, WP).transpose(1, 0, 2, 3)
    ).astype(f8)
    return {
        "x": xh,
        "b0": _b0().astype(ml_dtypes.bfloat16),
        "b1": _b1().astype(ml_dtypes.bfloat16),
        "eye": np.eye(128, dtype=ml_dtypes.bfloat16),
    }


def out_to_slab(yh: np.ndarray) -> np.ndarray:
    """[p, d, c, w] fp8 -> (d, H, W) float32 (h = 2p + c)."""
    d_out = yh.shape[1]
    return (
        yh.astype(np.float32).transpose(1, 0, 2, 3).reshape(d_out, H, W)
    )


def _np_dilate(vol: np.ndarray, ker: np.ndarray) -> np.ndarray:
    """Generic numpy fallback: conv3d(pad=1) > 0 for an arbitrary 3x3x3
    kernel (matches the reference exactly, including negative weights)."""
    b, ch, dd, hh, ww = vol.shape
    pad = np.pad(vol, ((0, 0), (0, 0), (1, 1), (1, 1), (1, 1)))
    kv = ker.reshape(3, 3, 3).astype(np.float64)
    s = np.zeros(vol.shape, np.float64)
    for i in range(3):
        for j in range(3):
            for k in range(3):
                if kv[i, j, k] != 0.0:
                    s += kv[i, j, k] * pad[:, :, i : i + dd, j : j + hh, k : k + ww]
    return (s > 0).astype(vol.dtype)


def kernel(binary_volume=None, kernel=None, **_unused):
    global _NC_CACHE, LAST_RESULTS
    vol = np.ascontiguousarray(np.asarray(binary_volume), dtype=np.float32)
    ker = np.asarray(kernel, dtype=np.float32)
    kv = ker.reshape(3, 3, 3)
    if (
        vol.shape != (B, 1, D_TOT, H, W)
        or not np.array_equal(kv != 0, _STAR)
        or not (kv[_STAR] > 0).all()
        or not ((vol == 0.0) | (vol == 1.0)).all()
    ):
        return _np_dilate(vol, ker).astype(np.asarray(binary_volume).dtype)

    from concourse.bass_utils import run_bass_kernel_spmd

    xr = vol.reshape(B, D_TOT, H, W)
    in_maps = []
    for core in range(N_CORES):
        b, s = divmod(core, D_SHARDS)
        d0 = s * D_OUT
        slab = np.zeros((D_IN, H, WP), np.float32)
        j_lo = 0 if d0 > 0 else 1                      # slab j <-> global d0-1+j
        j_hi = D_IN if d0 + D_OUT < D_TOT else D_IN - 1
        slab[j_lo:j_hi, :, 1 : W + 1] = xr[b, d0 - 1 + j_lo : d0 - 1 + j_hi]
        in_maps.append(host_inputs(slab))

    if _NC_CACHE is None:
        _NC_CACHE = build_nc()
    res = run_bass_kernel_spmd(_NC_CACHE, in_maps, list(range(N_CORES)), **RUN_KWARGS)
    LAST_RESULTS = res

    full = np.empty((B, 1, D_TOT, H, W), np.float32)
    for core in range(N_CORES):
        b, s = divmod(core, D_SHARDS)
        full[b, 0, s * D_OUT : (s + 1) * D_OUT] = out_to_slab(
            res.results[core]["y"]
        )
    return full



# revision 9
# speedup vs baseline: 3.2778x; 3.2778x over previous
"""Binary 3D dilation (star/6-connected structuring element) on 8 TRN2 cores.

out = (conv3d(x, star_kernel, pad=1) > 0)  for x in {0,1}^(2,1,256,256,256)

BIT-PACKED formulation: the volume is binary, so pack voxels into
uint32 words along W (host-side, free).  Words OVERLAP by 2 bits: word
j holds voxels w = 30j-1 .. 30j+30 (30 payload bits + 1 halo bit each
end), so the W-stencil never crosses a word boundary and the dilation
is a pure bitwise OR of 7 terms per packed word:

    out = C | (C<<1) | (C>>1)       # W-stencil (bits 0/31 are garbage,
        | X[d-1] | X[d+1]           #   discarded by the host unpack)
        | X[h-1] | X[h+1]           # D- and H-stencils

scalar_tensor_tensor fuses (shift | OR) in one DVE/Pool instruction,
so the whole chain is 6 instructions over ~1/28 the data of the float
formulation.  DMA traffic drops ~6x vs the fp8 baseline.  (The
no-overlap variant needs cross-word carry ops whose partial-word APs
are 4D — the walrus verifier limits ScalarTensorTensor to 3D APs.)

Sharding: core k -> batch k//4, D-quarter k%4 (64 output planes/core).
Partition layout: p = hb*8 + dq with hb in [0,16) blocks of 16 H-rows
and dq in [0,8) blocks of 8 D-planes.  Each partition holds its block
plus a 1-plane / 1-row halo on each side (host-duplicated, zero at
volume boundaries): X[p] = [10 planes, 18 rows, 8 words] uint32.
All stencil axes are then free-dim offsets within the partition.

Compute is split DVE / Pool by output-row ranges (independent chains).
"""

import sys

import numpy as np

if "/opt/trn_rl_repo" not in sys.path:
    sys.path.insert(0, "/opt/trn_rl_repo")

B = 2
D_TOT = 256
H = 256
W = 256
PAY = 30                  # payload bits per overlap-packed word
NW = 9                    # words per row: ceil(256/30) with 1-bit halos
N_CORES = 8
D_SHARDS = 4              # D split per batch entry
D_OUT = D_TOT // D_SHARDS           # 64 output planes per core

N_HB = 16                 # H blocks per core (partition dim)
RPP = H // N_HB           # 16 output rows per partition
N_DB = 8                  # D blocks per core (partition dim)
DPP = D_OUT // N_DB       # 8 output planes per partition
DL = DPP + 2              # input planes per partition (with halo)
RL = RPP + 2              # input rows per partition (with halo)

# 6-connected "star" structuring element mask (D,H,W offsets from center)
_STAR = np.zeros((3, 3, 3), bool)
_STAR[1, 1, 1] = _STAR[0, 1, 1] = _STAR[2, 1, 1] = True
_STAR[1, 0, 1] = _STAR[1, 2, 1] = True
_STAR[1, 1, 0] = _STAR[1, 1, 2] = True

# extra kwargs for run_bass_kernel_spmd (test.py sets trace=True here)
RUN_KWARGS: dict = {}
LAST_RESULTS = None


def build_nc(bpl: int = 4, dve_rows: int = 10):
    """Per-core Bass program (identical on all cores).

    bpl: output planes per compute block (pipeline granularity).
    dve_rows: of the 16 output rows per partition, how many DVE takes
    (the rest go to the Pool/gpsimd engine as an independent chain).
    """
    import concourse.bass as bass
    import concourse.mybir as mybir
    import concourse.tile as tile

    u32 = mybir.dt.uint32
    OR = mybir.AluOpType.bitwise_or
    SHL = mybir.AluOpType.logical_shift_left
    SHR = mybir.AluOpType.logical_shift_right

    assert DPP % bpl == 0
    n_blk = DPP // bpl

    nc = bass.Bass()
    x = nc.declare_dram_parameter("x", [128, DL, RL, NW], u32, isOutput=False)
    y = nc.declare_dram_parameter("y", [128, DPP, RPP, NW], u32, isOutput=True)

    with tile.TileContext(nc) as tc:
        with (
            tc.tile_pool(name="consts", bufs=1) as cpool,
            tc.tile_pool(name="data", bufs=1) as dpool,
        ):
            s1 = cpool.tile([128, 1], u32, tag="s1")
            nc.vector.memset(s1[:], 1)

            X = dpool.tile([128, DL, RL, NW], u32, tag="x")
            Y = dpool.tile([128, DPP, RPP, NW], u32, tag="y")

            # ---- input loads: chunked over planes, alternate SP/ACT ----
            # block k needs input planes [k*bpl, k*bpl + bpl + 2)
            load_engines = [nc.sync, nc.scalar]
            chunks = []
            hi0 = 0
            for k in range(n_blk):
                need = k * bpl + bpl + 2
                if need > hi0:
                    chunks.append((hi0, need))
                    hi0 = need
            for i, (lo, hi) in enumerate(chunks):
                eng = load_engines[i % 2]
                eng.dma_start(out=X[:, lo:hi], in_=x[:, lo:hi])

            # ---- compute: per block, independent row-chains ------------
            # (Pool/gpsimd cannot do 32-bit bitwise ops -- DVE only)
            row_splits = [
                (nc.vector, 0, RPP),
            ]
            for k in range(n_blk):
                a0 = k * bpl
                for eng, r0, r1 in row_splits:
                    if r0 >= r1:
                        continue
                    C = X[:, a0 + 1 : a0 + 1 + bpl, r0 + 1 : r1 + 1, :]
                    T = Y[:, a0 : a0 + bpl, r0:r1, :]
                    # W-stencil: shifts fused with OR accumulate (full word
                    # range -> rows x words coalesce, 3D APs as walrus wants)
                    eng.scalar_tensor_tensor(
                        out=T, in0=C, scalar=s1[:], in1=C, op0=SHL, op1=OR
                    )
                    eng.scalar_tensor_tensor(
                        out=T, in0=C, scalar=s1[:], in1=T, op0=SHR, op1=OR
                    )
                    # D-stencil
                    eng.tensor_tensor(
                        out=T,
                        in0=X[:, a0 : a0 + bpl, r0 + 1 : r1 + 1, :],
                        in1=T,
                        op=OR,
                    )
                    eng.tensor_tensor(
                        out=T,
                        in0=X[:, a0 + 2 : a0 + 2 + bpl, r0 + 1 : r1 + 1, :],
                        in1=T,
                        op=OR,
                    )
                    # H-stencil
                    eng.tensor_tensor(
                        out=T,
                        in0=X[:, a0 + 1 : a0 + 1 + bpl, r0:r1, :],
                        in1=T,
                        op=OR,
                    )
                    eng.tensor_tensor(
                        out=T,
                        in0=X[:, a0 + 1 : a0 + 1 + bpl, r0 + 2 : r1 + 2, :],
                        in1=T,
                        op=OR,
                    )
                # ---- store this block (alternate SP/ACT) ---------------
                store_eng = load_engines[(k + 1) % 2]
                store_eng.dma_start(out=y[:, a0 : a0 + bpl], in_=Y[:, a0 : a0 + bpl])

    import bass_rust as _bass_rust

    _bass_rust.generate_event_semaphores(nc)
    return nc


_NC_CACHE = None


def pack_volume(vol4: np.ndarray) -> np.ndarray:
    """(B, D, H, W) 0/1 float -> padded packed bits [B, D+2, H+2, NW] u32.

    Overlap packing: bit i of word j = voxel w = PAY*j - 1 + i, so each
    word carries its own 1-voxel W-halo and shifts never cross words.
    """
    bits = (vol4 != 0).astype(np.uint8)
    bb = np.zeros((B, D_TOT, H, PAY * NW + 2), np.uint8)  # w in [-1, 271)
    bb[..., 1 : W + 1] = bits
    win = np.lib.stride_tricks.sliding_window_view(bb, 32, axis=-1)
    win = win[..., :: PAY, :]                            # [B,D,H,NW,32]
    pk = np.packbits(win, axis=-1, bitorder="little")    # [B,D,H,NW,4] u8
    p32 = np.ascontiguousarray(pk).view(np.uint32)[..., 0]
    pad = np.zeros((B, D_TOT + 2, H + 2, NW), np.uint32)
    pad[:, 1:-1, 1:-1, :] = p32
    return pad


_DI = (DPP * np.arange(N_DB))[:, None] + np.arange(DL)   # [N_DB, DL]
_HI = (RPP * np.arange(N_HB))[:, None] + np.arange(RL)   # [N_HB, RL]


def core_input(ppad: np.ndarray, core: int) -> dict:
    """Per-core in_map from the padded packed volume."""
    b, q = divmod(core, D_SHARDS)
    sub = ppad[b, q * D_OUT : q * D_OUT + D_OUT + 2]     # [66, 258, NW]
    xd = sub[_DI]                                        # [N_DB, DL, 258, NW]
    xh = xd[:, :, _HI]                                   # [N_DB, DL, N_HB, RL, NW]
    X = np.ascontiguousarray(xh.transpose(2, 0, 1, 3, 4)).reshape(128, DL, RL, NW)
    return {"x": X}


def core_output(yh: np.ndarray) -> np.ndarray:
    """[128, DPP, RPP, NW] u32 -> (D_OUT, H, W) float32."""
    r = yh.reshape(N_HB, N_DB, DPP, RPP, NW).transpose(1, 2, 0, 3, 4)
    r = np.ascontiguousarray(r).view(np.uint8).reshape(D_OUT, H, NW * 4)
    bits = np.unpackbits(r, axis=-1, bitorder="little")  # [D_OUT, H, NW*32]
    bits = bits.reshape(D_OUT, H, NW, 32)[..., 1:31]     # drop halo bits
    return bits.reshape(D_OUT, H, NW * PAY)[..., :W].astype(np.float32)


def _np_dilate(vol: np.ndarray, ker: np.ndarray) -> np.ndarray:
    """Generic numpy fallback: conv3d(pad=1) > 0 for an arbitrary 3x3x3
    kernel (matches the reference exactly, including negative weights)."""
    b, ch, dd, hh, ww = vol.shape
    pad = np.pad(vol, ((0, 0), (0, 0), (1, 1), (1, 1), (1, 1)))
    kv = ker.reshape(3, 3, 3).astype(np.float64)
    s = np.zeros(vol.shape, np.float64)
    for i in range(3):
        for j in range(3):
            for k in range(3):
                if kv[i, j, k] != 0.0:
                    s += kv[i, j, k] * pad[:, :, i : i + dd, j : j + hh, k : k + ww]
    return (s > 0).astype(vol.dtype)


def kernel(binary_volume=None, kernel=None, **_unused):
    global _NC_CACHE, LAST_RESULTS
    vol = np.asarray(binary_volume)
    ker = np.asarray(kernel, dtype=np.float32)
    kv = ker.reshape(3, 3, 3)
    volf = np.ascontiguousarray(vol, dtype=np.float32)
    if (
        vol.shape != (B, 1, D_TOT, H, W)
        or not np.array_equal(kv != 0, _STAR)
        or not (kv[_STAR] > 0).all()
        or not ((volf == 0.0) | (volf == 1.0)).all()
    ):
        return _np_dilate(volf, ker).astype(vol.dtype)

    from concourse.bass_utils import run_bass_kernel_spmd

    ppad = pack_volume(volf.reshape(B, D_TOT, H, W))
    in_maps = [core_input(ppad, core) for core in range(N_CORES)]

    if _NC_CACHE is None:
        _NC_CACHE = build_nc()
    res = run_bass_kernel_spmd(_NC_CACHE, in_maps, list(range(N_CORES)), **RUN_KWARGS)
    LAST_RESULTS = res

    full = np.empty((B, 1, D_TOT, H, W), np.float32)
    for core in range(N_CORES):
        b, q = divmod(core, D_SHARDS)
        full[b, 0, q * D_OUT : (q + 1) * D_OUT] = core_output(res.results[core]["y"])
    return full


# revision 10
# speedup vs baseline: 3.3350x; 1.0174x over previous
"""Binary 3D dilation (star/6-connected structuring element) on 8 TRN2 cores.

out = (conv3d(x, star_kernel, pad=1) > 0)  for x in {0,1}^(2,1,256,256,256)

BIT-PACKED formulation: the volume is binary, so pack voxels into
uint32 words along W (host-side, free).  Words OVERLAP by 2 bits: word
j holds voxels w = 30j-1 .. 30j+30 (30 payload bits + 1 halo bit each
end), so the W-stencil never crosses a word boundary and the dilation
is a pure bitwise OR of 7 terms per packed word:

    out = C | (C<<1) | (C>>1)       # W-stencil (bits 0/31 are garbage,
        | X[d-1] | X[d+1]           #   discarded by the host unpack)
        | X[h-1] | X[h+1]           # D- and H-stencils

scalar_tensor_tensor fuses (shift | OR) in one DVE/Pool instruction,
so the whole chain is 6 instructions over ~1/28 the data of the float
formulation.  DMA traffic drops ~6x vs the fp8 baseline.  (The
no-overlap variant needs cross-word carry ops whose partial-word APs
are 4D — the walrus verifier limits ScalarTensorTensor to 3D APs.)

Sharding: core k -> batch k//4, D-quarter k%4 (64 output planes/core).
Partition layout: p = hb*8 + dq with hb in [0,16) blocks of 16 H-rows
and dq in [0,8) blocks of 8 D-planes.  Each partition holds its block
plus a 1-plane / 1-row halo on each side (host-duplicated, zero at
volume boundaries): X[p] = [10 planes, 18 rows, 8 words] uint32.
All stencil axes are then free-dim offsets within the partition.

Compute is split DVE / Pool by output-row ranges (independent chains).
"""

import sys

import numpy as np

if "/opt/trn_rl_repo" not in sys.path:
    sys.path.insert(0, "/opt/trn_rl_repo")

B = 2
D_TOT = 256
H = 256
W = 256
PAY = 30                  # payload bits per overlap-packed word
NW = 9                    # words per row: ceil(256/30) with 1-bit halos
N_CORES = 8
D_SHARDS = 4              # D split per batch entry
D_OUT = D_TOT // D_SHARDS           # 64 output planes per core

N_HB = 16                 # H blocks per core (partition dim)
RPP = H // N_HB           # 16 output rows per partition
N_DB = 8                  # D blocks per core (partition dim)
DPP = D_OUT // N_DB       # 8 output planes per partition
DL = DPP + 2              # input planes per partition (with halo)
RL = RPP + 2              # input rows per partition (with halo)

# 6-connected "star" structuring element mask (D,H,W offsets from center)
_STAR = np.zeros((3, 3, 3), bool)
_STAR[1, 1, 1] = _STAR[0, 1, 1] = _STAR[2, 1, 1] = True
_STAR[1, 0, 1] = _STAR[1, 2, 1] = True
_STAR[1, 1, 0] = _STAR[1, 1, 2] = True

# extra kwargs for run_bass_kernel_spmd (test.py sets trace=True here)
RUN_KWARGS: dict = {}
LAST_RESULTS = None


def build_nc(bpl: int = 2, dve_rows: int = 10):
    """Per-core Bass program (identical on all cores).

    bpl: output planes per compute block (pipeline granularity).
    dve_rows: of the 16 output rows per partition, how many DVE takes
    (the rest go to the Pool/gpsimd engine as an independent chain).
    """
    import concourse.bass as bass
    import concourse.mybir as mybir
    import concourse.tile as tile

    u32 = mybir.dt.uint32
    OR = mybir.AluOpType.bitwise_or
    SHL = mybir.AluOpType.logical_shift_left
    SHR = mybir.AluOpType.logical_shift_right

    assert DPP % bpl == 0
    n_blk = DPP // bpl

    nc = bass.Bass()
    x = nc.declare_dram_parameter("x", [128, DL, RL, NW], u32, isOutput=False)
    y = nc.declare_dram_parameter("y", [128, DPP, RPP, NW], u32, isOutput=True)

    with tile.TileContext(nc) as tc:
        with (
            tc.tile_pool(name="consts", bufs=1) as cpool,
            tc.tile_pool(name="data", bufs=1) as dpool,
        ):
            s1 = cpool.tile([128, 1], u32, tag="s1")
            nc.vector.memset(s1[:], 1)

            X = dpool.tile([128, DL, RL, NW], u32, tag="x")
            Y = dpool.tile([128, DPP, RPP, NW], u32, tag="y")

            # ---- input loads: chunked over planes, alternate SP/ACT ----
            # block k needs input planes [k*bpl, k*bpl + bpl + 2)
            load_engines = [nc.sync, nc.scalar]
            chunks = []
            hi0 = 0
            for k in range(n_blk):
                need = k * bpl + bpl + 2
                if need > hi0:
                    chunks.append((hi0, need))
                    hi0 = need
            for i, (lo, hi) in enumerate(chunks):
                eng = load_engines[i % 2]
                eng.dma_start(out=X[:, lo:hi], in_=x[:, lo:hi])

            # ---- compute: per block, independent row-chains ------------
            # (Pool/gpsimd cannot do 32-bit bitwise ops -- DVE only)
            row_splits = [
                (nc.vector, 0, RPP),
            ]
            for k in range(n_blk):
                a0 = k * bpl
                for eng, r0, r1 in row_splits:
                    if r0 >= r1:
                        continue
                    C = X[:, a0 + 1 : a0 + 1 + bpl, r0 + 1 : r1 + 1, :]
                    T = Y[:, a0 : a0 + bpl, r0:r1, :]
                    # W-stencil: shifts fused with OR accumulate (full word
                    # range -> rows x words coalesce, 3D APs as walrus wants)
                    eng.scalar_tensor_tensor(
                        out=T, in0=C, scalar=s1[:], in1=C, op0=SHL, op1=OR
                    )
                    eng.scalar_tensor_tensor(
                        out=T, in0=C, scalar=s1[:], in1=T, op0=SHR, op1=OR
                    )
                    # D-stencil
                    eng.tensor_tensor(
                        out=T,
                        in0=X[:, a0 : a0 + bpl, r0 + 1 : r1 + 1, :],
                        in1=T,
                        op=OR,
                    )
                    eng.tensor_tensor(
                        out=T,
                        in0=X[:, a0 + 2 : a0 + 2 + bpl, r0 + 1 : r1 + 1, :],
                        in1=T,
                        op=OR,
                    )
                    # H-stencil
                    eng.tensor_tensor(
                        out=T,
                        in0=X[:, a0 + 1 : a0 + 1 + bpl, r0:r1, :],
                        in1=T,
                        op=OR,
                    )
                    eng.tensor_tensor(
                        out=T,
                        in0=X[:, a0 + 1 : a0 + 1 + bpl, r0 + 2 : r1 + 2, :],
                        in1=T,
                        op=OR,
                    )
                # ---- store this block (alternate SP/ACT) ---------------
                store_eng = load_engines[(k + 1) % 2]
                store_eng.dma_start(out=y[:, a0 : a0 + bpl], in_=Y[:, a0 : a0 + bpl])

    import bass_rust as _bass_rust

    _bass_rust.generate_event_semaphores(nc)
    return nc


_NC_CACHE = None


def pack_volume(vol4: np.ndarray) -> np.ndarray:
    """(B, D, H, W) 0/1 float -> padded packed bits [B, D+2, H+2, NW] u32.

    Overlap packing: bit i of word j = voxel w = PAY*j - 1 + i, so each
    word carries its own 1-voxel W-halo and shifts never cross words.
    """
    bits = (vol4 != 0).astype(np.uint8)
    bb = np.zeros((B, D_TOT, H, PAY * NW + 2), np.uint8)  # w in [-1, 271)
    bb[..., 1 : W + 1] = bits
    win = np.lib.stride_tricks.sliding_window_view(bb, 32, axis=-1)
    win = win[..., :: PAY, :]                            # [B,D,H,NW,32]
    pk = np.packbits(win, axis=-1, bitorder="little")    # [B,D,H,NW,4] u8
    p32 = np.ascontiguousarray(pk).view(np.uint32)[..., 0]
    pad = np.zeros((B, D_TOT + 2, H + 2, NW), np.uint32)
    pad[:, 1:-1, 1:-1, :] = p32
    return pad


_DI = (DPP * np.arange(N_DB))[:, None] + np.arange(DL)   # [N_DB, DL]
_HI = (RPP * np.arange(N_HB))[:, None] + np.arange(RL)   # [N_HB, RL]


def core_input(ppad: np.ndarray, core: int) -> dict:
    """Per-core in_map from the padded packed volume."""
    b, q = divmod(core, D_SHARDS)
    sub = ppad[b, q * D_OUT : q * D_OUT + D_OUT + 2]     # [66, 258, NW]
    xd = sub[_DI]                                        # [N_DB, DL, 258, NW]
    xh = xd[:, :, _HI]                                   # [N_DB, DL, N_HB, RL, NW]
    X = np.ascontiguousarray(xh.transpose(2, 0, 1, 3, 4)).reshape(128, DL, RL, NW)
    return {"x": X}


def core_output(yh: np.ndarray) -> np.ndarray:
    """[128, DPP, RPP, NW] u32 -> (D_OUT, H, W) float32."""
    r = yh.reshape(N_HB, N_DB, DPP, RPP, NW).transpose(1, 2, 0, 3, 4)
    r = np.ascontiguousarray(r).view(np.uint8).reshape(D_OUT, H, NW * 4)
    bits = np.unpackbits(r, axis=-1, bitorder="little")  # [D_OUT, H, NW*32]
    bits = bits.reshape(D_OUT, H, NW, 32)[..., 1:31]     # drop halo bits
    return bits.reshape(D_OUT, H, NW * PAY)[..., :W].astype(np.float32)


def _np_dilate(vol: np.ndarray, ker: np.ndarray) -> np.ndarray:
    """Generic numpy fallback: conv3d(pad=1) > 0 for an arbitrary 3x3x3
    kernel (matches the reference exactly, including negative weights)."""
    b, ch, dd, hh, ww = vol.shape
    pad = np.pad(vol, ((0, 0), (0, 0), (1, 1), (1, 1), (1, 1)))
    kv = ker.reshape(3, 3, 3).astype(np.float64)
    s = np.zeros(vol.shape, np.float64)
    for i in range(3):
        for j in range(3):
            for k in range(3):
                if kv[i, j, k] != 0.0:
                    s += kv[i, j, k] * pad[:, :, i : i + dd, j : j + hh, k : k + ww]
    return (s > 0).astype(vol.dtype)


def kernel(binary_volume=None, kernel=None, **_unused):
    global _NC_CACHE, LAST_RESULTS
    vol = np.asarray(binary_volume)
    ker = np.asarray(kernel, dtype=np.float32)
    kv = ker.reshape(3, 3, 3)
    volf = np.ascontiguousarray(vol, dtype=np.float32)
    if (
        vol.shape != (B, 1, D_TOT, H, W)
        or not np.array_equal(kv != 0, _STAR)
        or not (kv[_STAR] > 0).all()
        or not ((volf == 0.0) | (volf == 1.0)).all()
    ):
        return _np_dilate(volf, ker).astype(vol.dtype)

    from concourse.bass_utils import run_bass_kernel_spmd

    ppad = pack_volume(volf.reshape(B, D_TOT, H, W))
    in_maps = [core_input(ppad, core) for core in range(N_CORES)]

    if _NC_CACHE is None:
        _NC_CACHE = build_nc()
    res = run_bass_kernel_spmd(_NC_CACHE, in_maps, list(range(N_CORES)), **RUN_KWARGS)
    LAST_RESULTS = res

    full = np.empty((B, 1, D_TOT, H, W), np.float32)
    for core in range(N_CORES):
        b, q = divmod(core, D_SHARDS)
        full[b, 0, q * D_OUT : (q + 1) * D_OUT] = core_output(res.results[core]["y"])
    return full
